# revision 7
# baseline (speedup 1.0000x reference)
"""BlockMultiHeadAttention on 8 TRN2 NeuronCores (Bass/Tile, single launch).

Sharding:
  Phase 1 (GroupLinear projections, the dominant cost -- 805 MB of weights):
    block-parallel. Core c owns blocks n in [8c, 8c+8) and projects q/k/v for
    ALL 32 batches with those blocks' weights. Each core reads only its
    ~100 MB weight shard; chip-wide the weights are read exactly once.
  Exchange: one AllToAll per projection redistributes [positions -> batches]:
    core c contributes [8 dest][8 n][4 b][1024 e]; after A2A it holds, for its
    4 batches, all 64 positions.
  Phase 2 (attention + fc + residual + LayerNorm): batch-parallel, core c owns
    batches [4c, 4c+4).

Host-side prep (inside kernel(), before launch): activation transposes for
matmul lhsT layout, fc_w transpose, mask -> additive bias (0 / -8e9, so
exp(0.125*(s+bias)) underflows to exactly 0 where masked), fc_b folded into
the residual, ln_g/ln_b replicated to [64, 1024] row tiles.
"""
import os
from contextlib import ExitStack

import numpy as np

H, DK, DV = 16, 64, 64
B, L, D = 32, 64, 1024
NC_CORES = 8
BPC = 8   # blocks per core
BB = 4    # batches per core

# float32r: fp32 storage, single-pass reduced-precision matmul (4x faster at
# N>=256 than true fp32). Toggle for the projection + fc matmuls.
USE_F32R = os.environ.get("BMHA_F32R", "1") == "1"

_BUILT = {}
LAST_RESULT = None  # BassKernelResults of the most recent kernel() call


def _build(use_f32r: bool):
    import concourse.bacc as bacc
    import concourse.tile as tile
    from concourse import mybir

    f32 = mybir.dt.float32
    f32r = mybir.dt.float32r
    wdt = f32r if use_f32r else f32   # dtype for fast-matmul operands
    AF = mybir.ActivationFunctionType
    ALU = mybir.AluOpType

    nc = bacc.Bacc("TRN2", target_bir_lowering=False, debug=False,
                   num_devices=NC_CORES)

    def inp(name, shape, dt=f32):
        return nc.dram_tensor(name, shape, dt, kind="ExternalInput").ap()

    xT = {p: inp(f"xT_{p}", [8, 128, BPC, B], wdt) for p in "qkv"}  # [kt,p,n,b]
    w = {p: inp(f"w_{p}", [BPC, D, D], wdt) for p in "qkv"}
    bias8 = inp("bias8", [BB, L, L])
    fc_wT = inp("fc_wT", [D, D], wdt)
    v_resid = inp("v_resid", [BB, L, D])
    ln_g_rep = inp("ln_g_rep", [L, D])
    ln_b_rep = inp("ln_b_rep", [L, D])
    ident = inp("ident", [128, 128])

    out_c = nc.dram_tensor("out_c", [BB, L, D], f32, kind="ExternalOutput").ap()
    attn_c = nc.dram_tensor("attn_c", [H, BB, L, L], f32,
                            kind="ExternalOutput").ap()

    with tile.TileContext(nc) as tc:
        with ExitStack() as ctx:
            _emit(ctx, tc, mybir, f32, wdt, AF, ALU, xT, w, bias8, fc_wT,
                  v_resid, ln_g_rep, ln_b_rep, ident, out_c, attn_c)
    nc.compile()
    return nc


def _emit(ctx, tc, mybir, f32, wdt, AF, ALU, xT, w, bias8, fc_wT, v_resid,
          ln_g_rep, ln_b_rep, ident, out_c, attn_c):
    nc = tc.nc

    const = ctx.enter_context(tc.tile_pool(name="const", bufs=1))
    ident_sb = const.tile([128, 128], f32)
    nc.sync.dma_start(ident_sb[:], ident[:])
    lng_sb = const.tile([L, D], f32)
    nc.sync.dma_start(lng_sb[:], ln_g_rep[:])
    lnb_sb = const.tile([L, D], f32)
    nc.sync.dma_start(lnb_sb[:], ln_b_rep[:])
    xt_sb = {}
    for p in "qkv":
        t = const.tile([128, 8, BPC, B], wdt, name=f"xt_{p}")  # [p,kt,n,b]
        nc.sync.dma_start(t[:], xT[p][:].rearrange("kt p n b -> p kt n b"))
        xt_sb[p] = t

    dram = ctx.enter_context(tc.tile_pool(name="dram", bufs=1, space="DRAM"))
    contribs, recvs = {}, {}
    for p in "qkv":
        contribs[p] = dram.tile([NC_CORES, BPC, BB, D], f32,
                                name=f"contrib_{p}")
        recvs[p] = dram.tile([NC_CORES, BPC, BB, D], f32, name=f"recv_{p}")

    # ---------------- phase 1: block-parallel projections ----------------
    wpool = ctx.enter_context(tc.tile_pool(name="wpool", bufs=2))
    evac = ctx.enter_context(tc.tile_pool(name="evac", bufs=2))
    ps1 = ctx.enter_context(tc.tile_pool(name="ps1", bufs=2, space="PSUM"))

    for p in "qkv":  # q, k first: phase-2 scores chain is the long pole
        for n in range(BPC):
            w_sb = wpool.tile([128, 8, D], wdt, tag="w", name=f"w_{p}{n}")
            nc.sync.dma_start(w_sb[:],
                              w[p][n].rearrange("(kt p2) e -> p2 kt e", p2=128))
            prj = evac.tile([B, D], f32, tag="prj", name=f"prj_{p}{n}")
            for ns in range(2):
                ps = ps1.tile([B, 512], f32, tag="ps1", name=f"ps_{p}{n}{ns}")
                for kt in range(8):
                    nc.tensor.matmul(ps[:], xt_sb[p][:, kt, n, :],
                                     w_sb[:, kt, ns * 512:(ns + 1) * 512],
                                     start=(kt == 0), stop=(kt == 7))
                nc.vector.tensor_copy(prj[:, ns * 512:(ns + 1) * 512], ps[:])
            nc.scalar.dma_start(contribs[p][:, n, :, :], prj[:])
        nc.gpsimd.collective_compute(
            "AllToAll", mybir.AluOpType.bypass,
            replica_groups=[list(range(NC_CORES))],
            ins=[contribs[p].opt()], outs=[recvs[p].opt()])

    # ---------------- phase 2: batch-parallel attention ------------------
    p2 = ctx.enter_context(tc.tile_pool(name="p2", bufs=2))
    p2s = ctx.enter_context(tc.tile_pool(name="p2s", bufs=3))
    ps2 = ctx.enter_context(tc.tile_pool(name="ps2", bufs=4, space="PSUM"))

    # fc_wT shares the weight pool slots (phase-1 weights are dead by now)
    fcw_sb = wpool.tile([128, 8, D], wdt, tag="w", name="fcw_sb")
    nc.sync.dma_start(fcw_sb[:], fc_wT[:].rearrange("(kt p2) d -> p2 kt d",
                                                    p2=128))

    for bl in range(BB):
        Qn = p2.tile([L, D], f32, tag="Qn", name=f"Qn{bl}")
        nc.scalar.dma_start(
            Qn[:], recvs["q"][:, :, bl, :].rearrange("c n e -> (c n) e"))
        Kn = p2.tile([L, D], f32, tag="Kn", name=f"Kn{bl}")
        nc.scalar.dma_start(
            Kn[:], recvs["k"][:, :, bl, :].rearrange("c n e -> (c n) e"))
        Vn = p2.tile([L, D], f32, tag="Vn", name=f"Vn{bl}")
        nc.scalar.dma_start(
            Vn[:], recvs["v"][:, :, bl, :].rearrange("c n e -> (c n) e"))
        bias_sb = p2.tile([L, L], f32, tag="bias", name=f"bias{bl}")
        nc.scalar.dma_start(bias_sb[:], bias8[bl])
        resid_sb = p2.tile([L, D], f32, tag="resid", name=f"resid{bl}")
        nc.scalar.dma_start(resid_sb[:], v_resid[bl])

        QT = p2.tile([128, 8, L], f32, tag="QT", name=f"QT{bl}")  # [p,et,i]
        KT = p2.tile([128, 8, L], f32, tag="KT", name=f"KT{bl}")
        for et in range(8):
            for src, dst in ((Qn, QT), (Kn, KT)):
                tp = ps2.tile([128, L], f32, tag="ps2", name=f"tp{bl}{et}")
                nc.tensor.transpose(tp[:], src[:, et * 128:(et + 1) * 128],
                                    ident_sb[:64, :64])
                nc.vector.tensor_copy(dst[:, et, :], tp[:])

        oT = p2.tile([128, 8, L], wdt, tag="oT", name=f"oT{bl}")  # [p,et,i]
        for h in range(H):
            pb = (h % 2) * 64
            ts = h // 2
            ps_s = ps2.tile([L, L], f32, tag="ps2", name=f"ps_s{bl}{h}")
            nc.tensor.matmul(ps_s[:], QT[pb:pb + 64, ts, :],
                             KT[pb:pb + 64, ts, :])
            sb_s = p2s.tile([L, L], f32, tag="sb_s", name=f"sb_s{bl}{h}")
            nc.vector.tensor_add(sb_s[:], ps_s[:], bias_sb[:])
            e_s = p2s.tile([L, L], f32, tag="e_s", name=f"e_s{bl}{h}")
            nc.scalar.activation(e_s[:], sb_s[:], AF.Exp, scale=0.125)
            rs = p2s.tile([L, 1], f32, tag="rs", name=f"rs{bl}{h}")
            nc.vector.reduce_sum(rs[:], e_s[:], axis=mybir.AxisListType.X)
            rr = p2s.tile([L, 1], f32, tag="rr", name=f"rr{bl}{h}")
            nc.vector.reciprocal(rr[:], rs[:])
            attn = p2s.tile([L, L], f32, tag="attn", name=f"attn{bl}{h}")
            nc.vector.tensor_scalar_mul(attn[:], e_s[:], rr[:])
            nc.scalar.dma_start(attn_c[h, bl], attn[:])
            tp2 = ps2.tile([L, L], f32, tag="ps2", name=f"tp2{bl}{h}")
            nc.tensor.transpose(tp2[:], attn[:], ident_sb[:64, :64])
            attnT = p2s.tile([L, L], f32, tag="attnT", name=f"attnT{bl}{h}")
            nc.vector.tensor_copy(attnT[:], tp2[:])
            ps_av = ps2.tile([L, L], f32, tag="ps2", name=f"ps_av{bl}{h}")
            nc.tensor.matmul(ps_av[:], Vn[:, h * 64:(h + 1) * 64], attnT[:])
            nc.vector.tensor_copy(oT[pb:pb + 64, ts, :], ps_av[:])

        x = p2.tile([L, D], f32, tag="x", name=f"x{bl}")
        for ds in range(2):
            ps_fc = ps2.tile([L, 512], f32, tag="ps2", name=f"ps_fc{bl}{ds}")
            for kt in range(8):
                nc.tensor.matmul(ps_fc[:], oT[:, kt, :],
                                 fcw_sb[:, kt, ds * 512:(ds + 1) * 512],
                                 start=(kt == 0), stop=(kt == 7))
            nc.vector.tensor_add(x[:, ds * 512:(ds + 1) * 512], ps_fc[:],
                                 resid_sb[:, ds * 512:(ds + 1) * 512])
        # LayerNorm over the free axis (in-place where safe)
        s1 = p2s.tile([L, 1], f32, tag="s1", name=f"s1{bl}")
        nc.vector.reduce_sum(s1[:], x[:], axis=mybir.AxisListType.X)
        mu = p2s.tile([L, 1], f32, tag="mu", name=f"mu{bl}")
        nc.vector.tensor_scalar_mul(mu[:], s1[:], 1.0 / D)
        nc.vector.tensor_scalar_sub(x[:], x[:], mu[:])          # x := x - mu
        sq = p2.tile([L, D], f32, tag="sq", name=f"sq{bl}")
        nc.vector.tensor_mul(sq[:], x[:], x[:])
        vs = p2s.tile([L, 1], f32, tag="vs", name=f"vs{bl}")
        nc.vector.reduce_sum(vs[:], sq[:], axis=mybir.AxisListType.X)
        var = p2s.tile([L, 1], f32, tag="var", name=f"var{bl}")
        nc.vector.tensor_scalar(var[:], vs[:], 1.0 / D, 1e-5, ALU.mult,
                                ALU.add)
        std = p2s.tile([L, 1], f32, tag="std", name=f"std{bl}")
        nc.scalar.sqrt(std[:], var[:])
        rstd = p2s.tile([L, 1], f32, tag="rstd", name=f"rstd{bl}")
        nc.vector.reciprocal(rstd[:], std[:])
        nc.vector.tensor_scalar_mul(x[:], x[:], rstd[:])        # x := xm*rstd
        nc.vector.tensor_mul(x[:], x[:], lng_sb[:])             # x := x*g
        nc.vector.tensor_add(x[:], x[:], lnb_sb[:])             # x := x+b
        nc.scalar.dma_start(out_c[bl], x[:])


def _get_nc(use_f32r: bool):
    if use_f32r not in _BUILT:
        _BUILT[use_f32r] = _build(use_f32r)
    return _BUILT[use_f32r]


def make_in_maps(q, k, v, mask, w_qs, w_ks, w_vs, fc_w, fc_b, ln_g, ln_b):
    f = np.float32
    q, k, v = (np.ascontiguousarray(np.asarray(a, f)) for a in (q, k, v))
    mask = np.asarray(mask)
    fc_wT = np.ascontiguousarray(np.asarray(fc_w, f).T)
    ident = np.eye(128, dtype=f)
    lng = np.ascontiguousarray(np.broadcast_to(np.asarray(ln_g, f), (L, D)))
    lnb = np.ascontiguousarray(np.broadcast_to(np.asarray(ln_b, f), (L, D)))
    fc_b = np.asarray(fc_b, f)
    in_maps = []
    for c in range(NC_CORES):
        pos = slice(BPC * c, BPC * (c + 1))
        bat = slice(BB * c, BB * (c + 1))
        im = {
            "bias8": np.where(mask[bat] > 0, f(0.0), f(-8e9)).astype(f),
            "fc_wT": fc_wT,
            "v_resid": np.ascontiguousarray(v[bat] + fc_b[None, None, :]),
            "ln_g_rep": lng, "ln_b_rep": lnb, "ident": ident,
        }
        for nm, x, wf in (("q", q, w_qs), ("k", k, w_ks), ("v", v, w_vs)):
            im[f"xT_{nm}"] = np.ascontiguousarray(
                x[:, pos, :].transpose(2, 1, 0)).reshape(8, 128, BPC, B)
            im[f"w_{nm}"] = np.ascontiguousarray(np.asarray(wf, f)[pos])
        in_maps.append(im)
    return in_maps


def kernel(q, k, v, mask, w_qs, w_ks, w_vs, fc_w, fc_b, ln_g, ln_b):
    global LAST_RESULT
    from concourse.bass_utils import run_bass_kernel_spmd

    in_maps = make_in_maps(q, k, v, mask, w_qs, w_ks, w_vs, fc_w, fc_b,
                           ln_g, ln_b)
    nc = _get_nc(USE_F32R)
    trace = os.environ.get("BMHA_TRACE", "0") == "1"
    res = run_bass_kernel_spmd(nc, in_maps, core_ids=list(range(NC_CORES)),
                               trace=trace)
    LAST_RESULT = res
    return assemble(res.results)


def assemble(results):
    out = np.zeros((B, L, D), np.float32)
    attn = np.zeros((H * B, L, L), np.float32)
    for c in range(NC_CORES):
        out[BB * c:BB * (c + 1)] = results[c]["out_c"]
        a = results[c]["attn_c"]
        for h in range(H):
            attn[h * B + BB * c: h * B + BB * (c + 1)] = a[h]
    return out, attn


# revision 16
# speedup vs baseline: 9.7716x; 9.7716x over previous
"""BlockMultiHeadAttention on 8 TRN2 NeuronCores (Bass/Tile, single launch).

Sharding:
  Phase 1 (GroupLinear projections, the dominant cost -- 805 MB of weights):
    block-parallel. Core c owns blocks n in [8c, 8c+8) and projects q/k/v for
    ALL 32 batches with those blocks' weights. Each core reads only its
    ~100 MB weight shard; chip-wide the weights are read exactly once.
  Exchange: one AllToAll per projection redistributes [positions -> batches]:
    core c contributes [8 dest][8 n][4 b][1024 e]; after A2A it holds, for its
    4 batches, all 64 positions.
  Phase 2 (attention + fc + residual + LayerNorm): batch-parallel, core c owns
    batches [4c, 4c+4).

Host-side prep (inside kernel(), before launch): activation transposes for
matmul lhsT layout, fc_w transpose, mask -> additive bias (0 / -8e9, so
exp(0.125*(s+bias)) underflows to exactly 0 where masked), fc_b folded into
the residual, ln_g/ln_b replicated to [64, 1024] row tiles.
"""
import os
from contextlib import ExitStack

import numpy as np

H, DK, DV = 16, 64, 64
B, L, D = 32, 64, 1024
NC_CORES = 8
BPC = 8   # blocks per core
BB = 4    # batches per core

# float32r: fp32 storage, single-pass reduced-precision matmul (4x faster at
# N>=256 than true fp32). Toggle for the projection + fc matmuls.
USE_F32R = os.environ.get("BMHA_F32R", "1") == "1"

_BUILT = {}
LAST_RESULT = None  # BassKernelResults of the most recent kernel() call


def _build(use_f32r: bool, reps: int = 1):
    import concourse.bacc as bacc
    import concourse.tile as tile
    from concourse import mybir

    f32 = mybir.dt.float32
    f32r = mybir.dt.float32r
    wdt = f32r if use_f32r else f32   # dtype for fast-matmul operands
    AF = mybir.ActivationFunctionType
    ALU = mybir.AluOpType

    nc = bacc.Bacc("TRN2", target_bir_lowering=False, debug=False,
                   num_devices=NC_CORES)

    def inp(name, shape, dt=f32):
        return nc.dram_tensor(name, shape, dt, kind="ExternalInput").ap()

    xT = {p: inp(f"xT_{p}", [8, 128, BPC, B], wdt) for p in "qkv"}  # [kt,p,n,b]
    w = {p: inp(f"w_{p}", [BPC, D, D], wdt) for p in "qkv"}
    bias8 = inp("bias8", [BB, L, L])
    fc_wT = inp("fc_wT", [D, D], wdt)
    v_resid = inp("v_resid", [BB, L, D])
    ln_g_rep = inp("ln_g_rep", [L, D])
    ln_b_rep = inp("ln_b_rep", [L, D])
    ident = inp("ident", [128, 128])
    # dummy input used by the timing harness to chain executions; zeros in
    # normal runs. Loaded to SBUF so it is a real NEFF input, never read.
    chain = inp("chain", [L, L])

    out_c = nc.dram_tensor("out_c", [BB, L, D], f32, kind="ExternalOutput").ap()
    attn_c = nc.dram_tensor("attn_c", [H, BB, L, L], f32,
                            kind="ExternalOutput").ap()

    with tile.TileContext(nc) as tc:
        with ExitStack() as octx:
            dram = octx.enter_context(
                tc.tile_pool(name="dram", bufs=1, space="DRAM"))
            contribs, recvs = {}, {}
            for p in "qkv":
                contribs[p] = dram.tile([NC_CORES, BPC, BB, D], f32,
                                        name=f"contrib_{p}")
                recvs[p] = dram.tile([NC_CORES, BPC, BB, D], f32,
                                     name=f"recv_{p}")
            for rep in range(reps):
                with ExitStack() as ctx:
                    _emit(ctx, tc, mybir, f32, wdt, AF, ALU, xT, w, bias8,
                          fc_wT, v_resid, ln_g_rep, ln_b_rep, ident, chain,
                          out_c, attn_c, contribs, recvs, rep)
    nc.compile()
    return nc


def _emit(ctx, tc, mybir, f32, wdt, AF, ALU, xT, w, bias8, fc_wT, v_resid,
          ln_g_rep, ln_b_rep, ident, chain, out_c, attn_c, contribs, recvs,
          rep):
    nc = tc.nc
    R = f"r{rep}"

    const = ctx.enter_context(tc.tile_pool(name=f"const{R}", bufs=1))
    ident_sb = const.tile([128, 128], f32, name=f"ident_sb{R}")
    nc.sync.dma_start(ident_sb[:], ident[:])
    chain_sb = const.tile([L, L], f32, name=f"chain_sb{R}")
    nc.sync.dma_start(chain_sb[:], chain[:])
    lng_sb = const.tile([L, D], f32, name=f"lng_sb{R}")
    nc.sync.dma_start(lng_sb[:], ln_g_rep[:])
    lnb_sb = const.tile([L, D], f32, name=f"lnb_sb{R}")
    nc.sync.dma_start(lnb_sb[:], ln_b_rep[:])
    xt_sb = {}
    for p in "qkv":
        t = const.tile([128, 8, BPC, B], wdt, name=f"xt_{p}{R}")  # [p,kt,n,b]
        nc.sync.dma_start(t[:], xT[p][:].rearrange("kt p n b -> p kt n b"))
        xt_sb[p] = t

    # ---------------- phase 1: block-parallel projections ----------------
    wpool = ctx.enter_context(tc.tile_pool(name=f"wpool{R}", bufs=2))
    evac = ctx.enter_context(tc.tile_pool(name=f"evac{R}", bufs=2))
    ps1 = ctx.enter_context(tc.tile_pool(name=f"ps1{R}", bufs=2, space="PSUM"))

    for p in "qkv":  # q, k first: phase-2 scores chain is the long pole
        for n in range(BPC):
            w_sb = wpool.tile([128, 8, D], wdt, tag="w", name=f"w_{p}{n}{R}")
            nc.sync.dma_start(w_sb[:],
                              w[p][n].rearrange("(kt p2) e -> p2 kt e", p2=128))
            prj = evac.tile([B, D], f32, tag="prj", name=f"prj_{p}{n}{R}")
            for ns in range(2):
                ps = ps1.tile([B, 512], f32, tag="ps1", name=f"ps_{p}{n}{ns}{R}")
                for kt in range(8):
                    nc.tensor.matmul(ps[:], xt_sb[p][:, kt, n, :],
                                     w_sb[:, kt, ns * 512:(ns + 1) * 512],
                                     start=(kt == 0), stop=(kt == 7))
                nc.vector.tensor_copy(prj[:, ns * 512:(ns + 1) * 512], ps[:])
            nc.scalar.dma_start(contribs[p][:, n, :, :], prj[:])
        nc.gpsimd.collective_compute(
            "AllToAll", mybir.AluOpType.bypass,
            replica_groups=[list(range(NC_CORES))],
            ins=[contribs[p].opt()], outs=[recvs[p].opt()])

    # ---------------- phase 2: batch-parallel attention ------------------
    p2 = ctx.enter_context(tc.tile_pool(name=f"p2{R}", bufs=2))
    p2s = ctx.enter_context(tc.tile_pool(name=f"p2s{R}", bufs=3))
    ps2 = ctx.enter_context(tc.tile_pool(name=f"ps2{R}", bufs=4, space="PSUM"))

    # fc_wT shares the weight pool slots (phase-1 weights are dead by now)
    fcw_sb = wpool.tile([128, 8, D], wdt, tag="w", name=f"fcw_sb{R}")
    nc.sync.dma_start(fcw_sb[:], fc_wT[:].rearrange("(kt p2) d -> p2 kt d",
                                                    p2=128))

    for bl in range(BB):
        Qn = p2.tile([L, D], f32, tag="Qn", name=f"Qn{bl}{R}")
        nc.scalar.dma_start(
            Qn[:], recvs["q"][:, :, bl, :].rearrange("c n e -> (c n) e"))
        Kn = p2.tile([L, D], f32, tag="Kn", name=f"Kn{bl}{R}")
        nc.scalar.dma_start(
            Kn[:], recvs["k"][:, :, bl, :].rearrange("c n e -> (c n) e"))
        Vn = p2.tile([L, D], f32, tag="Vn", name=f"Vn{bl}{R}")
        nc.scalar.dma_start(
            Vn[:], recvs["v"][:, :, bl, :].rearrange("c n e -> (c n) e"))
        bias_sb = p2.tile([L, L], f32, tag="bias", name=f"bias{bl}{R}")
        nc.scalar.dma_start(bias_sb[:], bias8[bl])
        resid_sb = p2.tile([L, D], f32, tag="resid", name=f"resid{bl}{R}")
        nc.scalar.dma_start(resid_sb[:], v_resid[bl])

        QT = p2.tile([128, 8, L], f32, tag="QT", name=f"QT{bl}{R}")  # [p,et,i]
        KT = p2.tile([128, 8, L], f32, tag="KT", name=f"KT{bl}{R}")
        for et in range(8):
            for src, dst in ((Qn, QT), (Kn, KT)):
                tp = ps2.tile([128, L], f32, tag="ps2", name=f"tp{bl}{et}{R}")
                nc.tensor.transpose(tp[:], src[:, et * 128:(et + 1) * 128],
                                    ident_sb[:64, :64])
                nc.vector.tensor_copy(dst[:, et, :], tp[:])

        oT = p2.tile([128, 8, L], wdt, tag="oT", name=f"oT{bl}{R}")  # [p,et,i]
        for h in range(H):
            pb = (h % 2) * 64
            ts = h // 2
            ps_s = ps2.tile([L, L], f32, tag="ps2", name=f"ps_s{bl}{h}{R}")
            nc.tensor.matmul(ps_s[:], QT[pb:pb + 64, ts, :],
                             KT[pb:pb + 64, ts, :])
            sb_s = p2s.tile([L, L], f32, tag="sb_s", name=f"sb_s{bl}{h}{R}")
            nc.vector.tensor_add(sb_s[:], ps_s[:], bias_sb[:])
            e_s = p2s.tile([L, L], f32, tag="e_s", name=f"e_s{bl}{h}{R}")
            nc.scalar.activation(e_s[:], sb_s[:], AF.Exp, scale=0.125)
            rs = p2s.tile([L, 1], f32, tag="rs", name=f"rs{bl}{h}{R}")
            nc.vector.reduce_sum(rs[:], e_s[:], axis=mybir.AxisListType.X)
            rr = p2s.tile([L, 1], f32, tag="rr", name=f"rr{bl}{h}{R}")
            nc.vector.reciprocal(rr[:], rs[:])
            attn = p2s.tile([L, L], f32, tag="attn", name=f"attn{bl}{h}{R}")
            nc.vector.tensor_scalar_mul(attn[:], e_s[:], rr[:])
            nc.scalar.dma_start(attn_c[h, bl], attn[:])
            tp2 = ps2.tile([L, L], f32, tag="ps2", name=f"tp2{bl}{h}{R}")
            nc.tensor.transpose(tp2[:], attn[:], ident_sb[:64, :64])
            attnT = p2s.tile([L, L], f32, tag="attnT", name=f"attnT{bl}{h}{R}")
            nc.vector.tensor_copy(attnT[:], tp2[:])
            ps_av = ps2.tile([L, L], f32, tag="ps2", name=f"ps_av{bl}{h}{R}")
            nc.tensor.matmul(ps_av[:], Vn[:, h * 64:(h + 1) * 64], attnT[:])
            nc.vector.tensor_copy(oT[pb:pb + 64, ts, :], ps_av[:])

        x = p2.tile([L, D], f32, tag="x", name=f"x{bl}{R}")
        for ds in range(2):
            ps_fc = ps2.tile([L, 512], f32, tag="ps2", name=f"ps_fc{bl}{ds}{R}")
            for kt in range(8):
                nc.tensor.matmul(ps_fc[:], oT[:, kt, :],
                                 fcw_sb[:, kt, ds * 512:(ds + 1) * 512],
                                 start=(kt == 0), stop=(kt == 7))
            nc.vector.tensor_add(x[:, ds * 512:(ds + 1) * 512], ps_fc[:],
                                 resid_sb[:, ds * 512:(ds + 1) * 512])
        # LayerNorm over the free axis (in-place where safe)
        s1 = p2s.tile([L, 1], f32, tag="s1", name=f"s1{bl}{R}")
        nc.vector.reduce_sum(s1[:], x[:], axis=mybir.AxisListType.X)
        mu = p2s.tile([L, 1], f32, tag="mu", name=f"mu{bl}{R}")
        nc.vector.tensor_scalar_mul(mu[:], s1[:], 1.0 / D)
        nc.vector.tensor_scalar_sub(x[:], x[:], mu[:])          # x := x - mu
        sq = p2.tile([L, D], f32, tag="sq", name=f"sq{bl}{R}")
        nc.vector.tensor_mul(sq[:], x[:], x[:])
        vs = p2s.tile([L, 1], f32, tag="vs", name=f"vs{bl}{R}")
        nc.vector.reduce_sum(vs[:], sq[:], axis=mybir.AxisListType.X)
        var = p2s.tile([L, 1], f32, tag="var", name=f"var{bl}{R}")
        nc.vector.tensor_scalar(var[:], vs[:], 1.0 / D, 1e-5, ALU.mult,
                                ALU.add)
        std = p2s.tile([L, 1], f32, tag="std", name=f"std{bl}{R}")
        nc.scalar.sqrt(std[:], var[:])
        rstd = p2s.tile([L, 1], f32, tag="rstd", name=f"rstd{bl}{R}")
        nc.vector.reciprocal(rstd[:], std[:])
        nc.vector.tensor_scalar_mul(x[:], x[:], rstd[:])        # x := xm*rstd
        nc.vector.tensor_mul(x[:], x[:], lng_sb[:])             # x := x*g
        nc.vector.tensor_add(x[:], x[:], lnb_sb[:])             # x := x+b
        nc.scalar.dma_start(out_c[bl], x[:])


def _get_nc(use_f32r: bool, reps: int = 1):
    key = (use_f32r, reps)
    if key not in _BUILT:
        _BUILT[key] = _build(use_f32r, reps)
    return _BUILT[key]


def make_in_maps(q, k, v, mask, w_qs, w_ks, w_vs, fc_w, fc_b, ln_g, ln_b):
    f = np.float32
    q, k, v = (np.ascontiguousarray(np.asarray(a, f)) for a in (q, k, v))
    mask = np.asarray(mask)
    fc_wT = np.ascontiguousarray(np.asarray(fc_w, f).T)
    ident = np.eye(128, dtype=f)
    lng = np.ascontiguousarray(np.broadcast_to(np.asarray(ln_g, f), (L, D)))
    lnb = np.ascontiguousarray(np.broadcast_to(np.asarray(ln_b, f), (L, D)))
    fc_b = np.asarray(fc_b, f)
    in_maps = []
    for c in range(NC_CORES):
        pos = slice(BPC * c, BPC * (c + 1))
        bat = slice(BB * c, BB * (c + 1))
        im = {
            "bias8": np.where(mask[bat] > 0, f(0.0), f(-8e9)).astype(f),
            "fc_wT": fc_wT,
            "v_resid": np.ascontiguousarray(v[bat] + fc_b[None, None, :]),
            "ln_g_rep": lng, "ln_b_rep": lnb, "ident": ident,
            "chain": np.zeros((L, L), f),
        }
        for nm, x, wf in (("q", q, w_qs), ("k", k, w_ks), ("v", v, w_vs)):
            im[f"xT_{nm}"] = np.ascontiguousarray(
                x[:, pos, :].transpose(2, 1, 0)).reshape(8, 128, BPC, B)
            im[f"w_{nm}"] = np.ascontiguousarray(np.asarray(wf, f)[pos])
        in_maps.append(im)
    return in_maps


def kernel(q, k, v, mask, w_qs, w_ks, w_vs, fc_w, fc_b, ln_g, ln_b):
    global LAST_RESULT
    from concourse.bass_utils import run_bass_kernel_spmd

    in_maps = make_in_maps(q, k, v, mask, w_qs, w_ks, w_vs, fc_w, fc_b,
                           ln_g, ln_b)
    nc = _get_nc(USE_F32R)
    trace = os.environ.get("BMHA_TRACE", "0") == "1"
    res = run_bass_kernel_spmd(nc, in_maps, core_ids=list(range(NC_CORES)),
                               trace=trace)
    LAST_RESULT = res
    return assemble(res.results)


def assemble(results):
    out = np.zeros((B, L, D), np.float32)
    attn = np.zeros((H * B, L, L), np.float32)
    for c in range(NC_CORES):
        out[BB * c:BB * (c + 1)] = results[c]["out_c"]
        a = results[c]["attn_c"]
        for h in range(H):
            attn[h * B + BB * c: h * B + BB * (c + 1)] = a[h]
    return out, attn
